# revision 26
# baseline (speedup 1.0000x reference)
"""Attentional Factorization Machine — Trainium2 Bass kernel (8 NeuronCores).

Sharding: data-parallel over batch (2048/8 = 256 per core, as 2 groups of 128,
each group = two 64-batch halves packed on SBUF partition halves).

v2 dataflow per 128-batch group:
  1. Per-field indirect-DMA gather of the augmented table [500000, 65]
     (= emb ++ lin_w) into batch-major rows gt [128, 50*65] fp32.
  2. PE transposes (fp32 in, bf16 drain) -> fact [128 part = d(x2 halves),
     50*64 = (field, batch)] bf16.
  3. Per pair-block i (j-outer, batch-inner): inter = f_i bcast * f_j on
     DVE/Pool in bf16 -> [128, W*64].
  4. mm1: ONE block-diag bf16 matmul [K=128, M=128] per 512-chunk
     (stationary diag(W1', W1'), W1' = W1*diag|w2|) -> hp PSUM fp32.
     Drain (bias+ReLU, fp32->bf16) rotates over ACT/DVE.
     mmS: block-diag ones [128, 2] over inter -> intersum strips.
     mm2 (lagged 1 chunk): block-diag sign(w2) [128, 2] over hs -> logit strips.
  5. Strips DMA'd contiguously from PSUM into stack tiles
     [row i | row 64+i, (j-1)*64+b] (lgt_stk pre-set to -20, one_stk to 0,
     once; pad cols never rewritten).
  6. Finale per group: exp on ACT, grouped reduces over j on DVE,
     partition-reduce over i via tiny ones-matmuls, 1-lane div, plus the
     linear term reduced from the gathered lin_w column.
"""

import sys

for _p in ("/opt/trn_rl_repo",):
    if _p not in sys.path:
        sys.path.insert(0, _p)

import numpy as np

import concourse.bass as bass
from concourse import bacc
import concourse.mybir as mybir
from concourse.tile import TileContext
from concourse.masks import make_identity
from concourse.bass_utils import run_bass_kernel_spmd

F = 50
D = 64
CARD = 10000
B = 2048
NCORES = 8
BPC = B // NCORES          # 256 batches per core
G = 2                      # groups of 128 per core
NB = F - 1                 # 49 pair blocks / j values per row
P = F * (F - 1) // 2       # 1225 pairs
ROWD = D + 1               # augmented row width (emb ++ lin_w)
SW = NB * D                # stack width = 3136
FP32 = mybir.dt.float32
BF16 = mybir.dt.bfloat16
CHUNK = 512                # cols per matmul chunk (1 PSUM bank of fp32)
CJ = CHUNK // D            # j's per chunk = 8
DBG = True


def build_nc():
    nc = bacc.Bacc(None, target_bir_lowering=False)

    idx_d = nc.dram_tensor("idx", [BPC, F], mybir.dt.int32, kind="ExternalInput")
    tab_d = nc.dram_tensor("tab", [CARD * F, ROWD], FP32, kind="ExternalInput")
    w1p_d = nc.dram_tensor("w1p", [D, D], FP32, kind="ExternalInput")
    sgn_d = nc.dram_tensor("sgn", [D, 1], FP32, kind="ExternalInput")
    b1p_d = nc.dram_tensor("b1p", [D, 1], FP32, kind="ExternalInput")
    linb_d = nc.dram_tensor("linb", [128, 1], FP32, kind="ExternalInput")
    y_d = nc.dram_tensor("y", [BPC, 1], FP32, kind="ExternalOutput")
    if DBG:
        dbg_lgt = nc.dram_tensor("dbg_lgt", [128, SW], FP32, kind="ExternalOutput")
        dbg_one = nc.dram_tensor("dbg_one", [128, SW], FP32, kind="ExternalOutput")
        dbg_rd = nc.dram_tensor("dbg_rd", [128, 128], FP32, kind="ExternalOutput")

    with TileContext(nc) as tc:
        with (
            tc.tile_pool(name="const", bufs=1) as cpool,
            tc.tile_pool(name="gath", bufs=2) as gpool,
            tc.tile_pool(name="fact", bufs=2) as fpool,
            tc.tile_pool(name="inter", bufs=2) as ipool,
            tc.tile_pool(name="hs", bufs=5) as hpool,
            tc.tile_pool(name="stg", bufs=3) as stgpool,
            tc.tile_pool(name="small", bufs=4) as smpool,
            tc.tile_pool(name="tp", bufs=1, space="PSUM") as ptp,
            tc.tile_pool(name="hp", bufs=2, space="PSUM") as php,
            tc.tile_pool(name="sp", bufs=2, space="PSUM") as psp,
            tc.tile_pool(name="spr", bufs=1, space="PSUM") as pspr,
        ):
            # ---------------- constants / weights prep ----------------
            ident = cpool.tile([128, 128], FP32)
            make_identity(nc, ident[:])
            spr_t = pspr.tile([64, 128], FP32, tag="spr")
            nc.tensor.transpose(
                spr_t[0:64, 0:64], ident[0:64, 0:64], ident[0:64, 0:64]
            )

            idx_sb = cpool.tile([128, G * F], mybir.dt.int32)
            nc.sync.dma_start(
                out=idx_sb[:].rearrange("p (g f) -> p g f", g=G),
                in_=idx_d[:].rearrange("(g p) f -> p g f", g=G),
            )

            # fp32 staging for weights
            w1f = cpool.tile([64, D], FP32)
            nc.sync.dma_start(out=w1f[:], in_=w1p_d[:])
            sgnf = cpool.tile([64, 1], FP32)
            nc.sync.dma_start(out=sgnf[:], in_=sgn_d[:])

            # block-diagonal bf16 stationaries
            w1bd = cpool.tile([128, 128], BF16)
            nc.vector.memset(w1bd[:], 0.0)
            nc.scalar.activation(
                out=w1bd[0:64, 0:64], in_=w1f[:],
                func=mybir.ActivationFunctionType.Copy,
            )
            nc.sync.dma_start(out=w1bd[64:128, 64:128], in_=w1bd[0:64, 0:64])

            sgn_bd = cpool.tile([128, 2], BF16)
            nc.vector.memset(sgn_bd[:], 0.0)
            nc.scalar.activation(
                out=sgn_bd[0:64, 0:1], in_=sgnf[:],
                func=mybir.ActivationFunctionType.Copy,
            )
            nc.sync.dma_start(out=sgn_bd[64:128, 1:2], in_=sgn_bd[0:64, 0:1])

            ones_bd = cpool.tile([128, 2], BF16)
            nc.vector.memset(ones_bd[:], 0.0)
            nc.vector.memset(ones_bd[0:64, 0:1], 1.0)
            nc.vector.memset(ones_bd[64:128, 1:2], 1.0)

            b1bd = cpool.tile([128, 1], FP32)
            nc.sync.dma_start(out=b1bd[0:64, :], in_=b1p_d[:])
            nc.sync.dma_start(out=b1bd[64:128, :], in_=b1bd[0:64, :])

            onesP = cpool.tile([128, 1], FP32)
            nc.vector.memset(onesP[:], 1.0)

            linb = cpool.tile([128, 1], FP32)
            nc.sync.dma_start(out=linb[:], in_=linb_d[:])

            # stacks: one-time pad init; valid cols are rewritten each group
            lgt_stk = cpool.tile([128, SW], FP32)
            nc.gpsimd.memset(lgt_stk[:], -20.0)
            one_stk = cpool.tile([128, SW], FP32)
            nc.gpsimd.memset(one_stk[:], 0.0)
            e_stk = cpool.tile([128, SW], FP32)
            rd = cpool.tile([128, 128], FP32)

            # ---------------- main loop over 128-batch groups ----------------
            nchunk = 0   # global chunk counter for drain rotation
            nprod = 0    # global product counter for engine rotation
            for g in range(G):
                gt = gpool.tile([128, F * ROWD], FP32, tag="gt")
                for f in range(F):
                    nc.gpsimd.indirect_dma_start(
                        out=gt[:, f * ROWD:(f + 1) * ROWD],
                        out_offset=None,
                        in_=tab_d[:],
                        in_offset=bass.IndirectOffsetOnAxis(
                            ap=idx_sb[:, g * F + f:g * F + f + 1], axis=0
                        ),
                    )
                gt3 = gt[:].rearrange("p (f e) -> p f e", e=ROWD)

                # linear term: sum over fields of the gathered lin_w column
                lin_t = smpool.tile([128, 1], FP32, tag="lint")
                nc.vector.tensor_reduce(
                    out=lin_t[:], in_=gt3[:, :, D:ROWD].rearrange("p f e -> p (f e)"),
                    axis=mybir.AxisListType.X, op=mybir.AluOpType.add,
                )
                nc.vector.tensor_tensor(
                    out=lin_t[:], in0=lin_t[:], in1=linb[:], op=mybir.AluOpType.add
                )

                # factors^T: [d (x2 halves), (field, batch)] in bf16
                fact = fpool.tile([128, F * D], BF16, tag="fact")
                tmpb = fpool.tile([64, F * D], BF16, tag="tmpb")
                for fb in range(0, F, 8):
                    nf = min(8, F - fb)
                    tpa = ptp.tile([64, 8, D], FP32, tag="tpa")
                    tpb = ptp.tile([64, 8, D], FP32, tag="tpb")
                    for j in range(nf):
                        f = fb + j
                        nc.tensor.transpose(
                            tpa[:, j, :],
                            gt3[0:64, f, 0:D],
                            ident[0:64, 0:64],
                        )
                        nc.tensor.transpose(
                            tpb[:, j, :],
                            gt3[64:128, f, 0:D],
                            ident[64:128, 64:128],
                        )
                    nc.scalar.activation(
                        out=fact[0:64, fb * D:(fb + nf) * D],
                        in_=tpa[:, 0:nf, :],
                        func=mybir.ActivationFunctionType.Copy,
                    )
                    nc.vector.tensor_copy(
                        tmpb[:, fb * D:(fb + nf) * D], tpb[:, 0:nf, :]
                    )
                nc.sync.dma_start(out=fact[64:128, :], in_=tmpb[:])

                # pair blocks: j-outer, batch-inner
                state = {
                    "pend": [],       # [(stile, hs, coff, N, i)] up to LAG
                    "nstrip": 0,
                }
                LAG = 2

                def emit_pend(force=False):
                    """Emit the lagged mm2 for a previous chunk, then copy
                    its strip tile to SBUF and DMA the strips to the stacks."""
                    if not state["pend"] or (not force and len(state["pend"]) < LAG):
                        return
                    stile, phs_t, poff, pn, pi = state["pend"].pop(0)
                    nc.tensor.matmul(
                        stile[32:34, 0:pn],
                        sgn_bd[:], phs_t[:, 0:pn],
                        start=True, stop=True, tile_position=(0, 32),
                    )
                    stg = stgpool.tile([34, CHUNK], FP32, tag="stg")
                    if state["nstrip"] % 3 != 1:
                        nc.scalar.activation(
                            out=stg[:, 0:pn], in_=stile[0:34, 0:pn],
                            func=mybir.ActivationFunctionType.Copy,
                        )
                    else:
                        nc.vector.tensor_copy(stg[:, 0:pn], stile[0:34, 0:pn])
                    state["nstrip"] += 1
                    nc.sync.dma_start(
                        out=one_stk[pi:pi + 1, poff:poff + pn],
                        in_=stg[0:1, 0:pn])
                    nc.sync.dma_start(
                        out=one_stk[64 + pi:65 + pi, poff:poff + pn],
                        in_=stg[1:2, 0:pn])
                    nc.sync.dma_start(
                        out=lgt_stk[pi:pi + 1, poff:poff + pn],
                        in_=stg[32:33, 0:pn])
                    nc.sync.dma_start(
                        out=lgt_stk[64 + pi:65 + pi, poff:poff + pn],
                        in_=stg[33:34, 0:pn])

                for i in range(NB):
                    W = NB - i
                    blk = W * D
                    inter = ipool.tile([128, blk], BF16, tag="inter")
                    peng = (nc.vector, nc.vector, nc.vector)[nprod % 3]
                    nprod += 1
                    peng.tensor_tensor(
                        out=inter[:].rearrange("p (j b) -> p j b", b=D),
                        in0=fact[:, i * D:(i + 1) * D]
                        .rearrange("p (o b) -> p o b", o=1)
                        .to_broadcast([128, W, D]),
                        in1=fact[:, (i + 1) * D:F * D]
                        .rearrange("p (j b) -> p j b", b=D),
                        op=mybir.AluOpType.mult,
                    )

                    for c0 in range(0, W, CJ):
                        nj = min(CJ, W - c0)
                        N = nj * D
                        coff = (i + c0) * D   # stack column offset
                        hp = php.tile([128, CHUNK], FP32, tag="hp")
                        stile = psp.tile([34, CHUNK], FP32, tag="sp", name="stile")
                        nc.tensor.matmul(
                            hp[:, 0:N], w1bd[:], inter[:, c0 * D:c0 * D + N],
                            start=True, stop=True, tile_position=(0, 0),
                        )
                        nc.tensor.matmul(
                            stile[0:2, 0:N],
                            ones_bd[:], inter[:, c0 * D:c0 * D + N],
                            start=True, stop=True, tile_position=(0, 0),
                        )
                        hs = hpool.tile([128, CHUNK], BF16, tag="hs")
                        deng = (nc.scalar, nc.vector, nc.scalar)[nchunk % 3]
                        nchunk += 1
                        if deng is nc.scalar:
                            nc.scalar.activation(
                                out=hs[:, 0:N], in_=hp[:, 0:N],
                                func=mybir.ActivationFunctionType.Relu,
                                bias=b1bd[:, 0:1],
                            )
                        else:
                            nc.vector.tensor_scalar(
                                out=hs[:, 0:N], in0=hp[:, 0:N],
                                scalar1=b1bd[:, 0:1], scalar2=0.0,
                                op0=mybir.AluOpType.add, op1=mybir.AluOpType.max,
                            )
                        # lagged mm2 + strip flush for an earlier chunk
                        emit_pend()
                        state["pend"].append((stile, hs, coff, N, i))

                # flush remaining chunks of the group
                while state["pend"]:
                    emit_pend(force=True)

                # ---------------- group finale ----------------
                if DBG and g == 0:
                    nc.sync.dma_start(out=dbg_lgt[:], in_=lgt_stk[:])
                    nc.sync.dma_start(out=dbg_one[:], in_=one_stk[:])
                nc.scalar.activation(
                    out=e_stk[:], in_=lgt_stk[:],
                    func=mybir.ActivationFunctionType.Exp,
                )
                # den = sum_j e  (view cols as [b (stride 1), j (stride 64)])
                nc.vector.tensor_reduce(
                    out=rd[:, 0:64],
                    in_=e_stk[:].rearrange("p (j b) -> p b j", b=D),
                    axis=mybir.AxisListType.X, op=mybir.AluOpType.add,
                )
                # w = e * one (in place), then num = sum_j w
                nc.gpsimd.tensor_tensor(
                    out=e_stk[:], in0=e_stk[:], in1=one_stk[:],
                    op=mybir.AluOpType.mult,
                )
                nc.vector.tensor_reduce(
                    out=rd[:, 64:128],
                    in_=e_stk[:].rearrange("p (j b) -> p b j", b=D),
                    axis=mybir.AxisListType.X, op=mybir.AluOpType.add,
                )
                if DBG and g == 0:
                    nc.sync.dma_start(out=dbg_rd[:], in_=rd[:])
                # partition reduce over i via tiny matmuls
                spr = pspr.tile([64, 128], FP32, tag="spr")
                nc.tensor.matmul(
                    spr[0:1, 0:128], onesP[0:NB, 0:1], rd[0:NB, :],
                    start=True, stop=True, tile_position=(0, 0),
                )
                nc.tensor.matmul(
                    spr[32:33, 0:128], onesP[64:64 + NB, 0:1], rd[64:64 + NB, :],
                    start=True, stop=True, tile_position=(64, 32),
                )
                # attended = num / den per half (1-lane ops, tiny)
                rz = smpool.tile([34, 64], FP32, tag="rz")
                att = smpool.tile([34, 64], FP32, tag="att")
                nc.vector.reciprocal(rz[0:1, :], spr[0:1, 0:64])
                nc.vector.reciprocal(rz[32:33, :], spr[32:33, 0:64])
                nc.vector.tensor_tensor(
                    out=att[0:1, :], in0=spr[0:1, 64:128], in1=rz[0:1, :],
                    op=mybir.AluOpType.mult,
                )
                nc.vector.tensor_tensor(
                    out=att[32:33, :], in0=spr[32:33, 64:128], in1=rz[32:33, :],
                    op=mybir.AluOpType.mult,
                )
                yg = smpool.tile([128, 1], FP32, tag="yg")
                nc.sync.dma_start(out=yg[0:64, 0:1], in_=att[0:1, :])
                nc.sync.dma_start(out=yg[64:128, 0:1], in_=att[32:33, :])
                nc.vector.tensor_tensor(
                    out=yg[:], in0=yg[:], in1=lin_t[:], op=mybir.AluOpType.add
                )
                nc.sync.dma_start(out=y_d[g * 128:(g + 1) * 128, :], in_=yg[:])

    nc.compile()
    return nc


_CACHE = {}


def kernel(x, emb, W1, b1, w2, b2, lin_w, lin_b):
    x = np.asarray(x)
    emb = np.asarray(emb, dtype=np.float32)
    W1 = np.asarray(W1, dtype=np.float32)
    b1 = np.asarray(b1, dtype=np.float32)
    w2 = np.asarray(w2, dtype=np.float32)
    lin_w = np.asarray(lin_w, dtype=np.float32)
    lin_b = np.asarray(lin_b, dtype=np.float32)

    # host-side input staging (layout only): global row ids + augmented table
    idx = (x.astype(np.int64) + (np.arange(F, dtype=np.int64) * CARD)[None, :])
    idx = idx.astype(np.int32)
    tab = np.concatenate([emb, lin_w.reshape(-1, 1)], axis=1).astype(np.float32)
    tab = np.ascontiguousarray(tab)
    aw2 = np.abs(w2.reshape(-1))
    w1p = np.ascontiguousarray((W1 * aw2[None, :]).astype(np.float32))
    b1p = np.ascontiguousarray((b1.reshape(-1) * aw2).reshape(D, 1).astype(np.float32))
    sgn = np.ascontiguousarray(np.sign(w2.reshape(D, 1)).astype(np.float32))
    linb = np.broadcast_to(
        lin_b.reshape(1, 1), (128, 1)
    ).astype(np.float32).copy()

    if "nc" not in _CACHE:
        _CACHE["nc"] = build_nc()
    nc = _CACHE["nc"]

    in_maps = []
    for c in range(NCORES):
        in_maps.append({
            "idx": np.ascontiguousarray(idx[c * BPC:(c + 1) * BPC]),
            "tab": tab,
            "w1p": w1p,
            "sgn": sgn,
            "b1p": b1p,
            "linb": linb,
        })

    _CACHE["last_in_maps"] = in_maps
    res = run_bass_kernel_spmd(nc, in_maps, core_ids=list(range(NCORES)))
    outs = [res.results[c]["y"] for c in range(NCORES)]
    _CACHE["last_res"] = res
    return np.concatenate(outs, axis=0).astype(np.float32)


if __name__ == "__main__":
    sys.path.insert(0, "/root/problem")
    import reference

    inputs = {k: np.asarray(v) for k, v in reference.setup_inputs().items()}
    y = kernel(**inputs)
    print(y.shape, y.dtype, y[:4, 0])
